# revision 4
# baseline (speedup 1.0000x reference)
"""ARB loss kernel for Trainium2, SPMD across 8 NeuronCores.

Reference computation (n=8192 rows, C=32000 classes):
    counts = bincount(y, C)                       # label histogram
    w[i]   = counts[y[i]]
    rowsum[i] = sum_c output[i, c]
    denom[i]  = (n / w[i]) * rowsum[i]
    loss = -mean_i log(output[i, y[i]] / denom[i])
         = log(n) - (1/n) * sum_i log(output[i,y[i]] * w[i] / rowsum[i])

Sharding: data-parallel over rows, 1024 rows per core. Each core:
  - streams its 1024x32000 f32 shard (131 MB) through SBUF in
    [128 x 8000] tiles; row sums are computed on the fly, split between
    the Vector engine (reduce_sum over cols [0:5504)) and the Scalar
    engine (activation Copy + accum_out over cols [5504:8000)) so
    neither engine is the bottleneck — the kernel is HBM-DMA bound.
  - computes w for its rows from the full label vector (replicated to
    every core, so no bincount all-reduce is needed): per 128-row block,
    tensor_scalar(is_equal) against the 8192-long label list with a
    fused add-reduction.
  - gathers output[i, y[i]] with elementwise indirect DMA.
  - evaluates log(true*w/rowsum) on the Scalar engine with a fused
    free-dim accumulation, yielding one partial sum per partition.
Host unshard: loss = log(n) - sum(all per-partition partials)/n.
"""

import math
import sys
from contextlib import ExitStack

import numpy as np

if "/opt/trn_rl_repo" not in sys.path:
    sys.path.insert(0, "/opt/trn_rl_repo")

# bass_utils imports antenv.axon_hooks when BASS_TRACE is set; make sure a
# stub exists so a missing module never crashes the run (trace then simply
# degrades to no-profile).
try:
    import antenv.axon_hooks  # noqa: F401
except ImportError:
    import types

    try:
        import antenv

        _stub = types.ModuleType("antenv.axon_hooks")
        _stub._HOOK = None
        _stub.set_axon_ntff_profile_hook = lambda h: setattr(_stub, "_HOOK", h)
        _stub.get_axon_ntff_profile_hook = lambda: _stub._HOOK
        sys.modules["antenv.axon_hooks"] = _stub
        antenv.axon_hooks = _stub
    except ImportError:
        pass

N = 8192           # total rows
C = 32000          # classes
NCORES = 8
RPC = N // NCORES  # rows per core = 1024
P = 128            # partitions
RB = RPC // P      # row blocks per core = 8
COLCH = 8000       # columns per streamed tile
NCH = C // COLCH   # column chunks per row block = 4
NT = RB * NCH      # streamed tiles per core = 32
NBUF = 3           # stream buffers
D_DVE = 5504       # columns of each tile reduced on VectorE
# remaining COLCH - D_DVE columns reduced on ScalarE

_CACHE = {}


def _build_nc():
    import concourse.bass as bass
    import concourse.mybir as mybir

    f32 = mybir.dt.float32
    i32 = mybir.dt.int32
    bf16 = mybir.dt.bfloat16
    D_ACT = COLCH - D_DVE

    nc = bass.Bass()
    x_ext = nc.dram_tensor("x", [RPC, C], f32, kind="ExternalInput")
    yf_ext = nc.dram_tensor("yf", [1, N], f32, kind="ExternalInput")
    ylf_ext = nc.dram_tensor("ylf", [P, RB], f32, kind="ExternalInput")
    off_ext = nc.dram_tensor("off", [P, RB], i32, kind="ExternalInput")
    out_ext = nc.dram_tensor("out", [P, 2], f32, kind="ExternalOutput")

    with ExitStack() as es:
        ec = es.enter_context
        data = [
            ec(nc.sbuf_tensor(f"data{j}", [P, COLCH], f32))
            for j in range(NBUF)
        ]
        yfb = ec(nc.sbuf_tensor([P, N], f32))
        eqscr = ec(nc.sbuf_tensor([P, N], bf16))
        act_scr = ec(nc.sbuf_tensor([P, D_ACT], f32))
        rs_part = ec(nc.sbuf_tensor([P, NT], f32))   # DVE partials per tile
        act_part = ec(nc.sbuf_tensor([P, NT], f32))  # ACT partials per tile
        sum32 = ec(nc.sbuf_tensor([P, NT], f32))
        rs = ec(nc.sbuf_tensor([P, RB], f32))
        w_sb = ec(nc.sbuf_tensor([P, RB], f32))
        tv = ec(nc.sbuf_tensor([P, RB], f32))
        ylf_sb = ec(nc.sbuf_tensor([P, RB], f32))
        off_sb = ec(nc.sbuf_tensor([P, RB], i32))
        tprod = ec(nc.sbuf_tensor([P, RB], f32))
        logt = ec(nc.sbuf_tensor([P, RB], f32))
        acc = ec(nc.sbuf_tensor([P, 2], f32))

        dmaL = [ec(nc.semaphore(f"dmaL{j}")) for j in range(NBUF)]
        dmaP = ec(nc.semaphore("dmaP"))
        dmaG = ec(nc.semaphore("dmaG"))
        vsem = ec(nc.semaphore("vsem"))
        asem = ec(nc.semaphore("asem"))
        block = ec(nc.Block())

        # --- precompute the DVE instruction schedule so producers know the
        # vsem value at which each tile's reduce has completed.
        # DVE order: tile reduces 0..31, with count op b inserted after
        # tile 4b+3's reduce.
        v_done = [0] * NT
        v = 0
        for i in range(NT):
            v += 1            # reduce of tile i
            v_done[i] = v
            if i % 4 == 3:
                v += 1        # count op for block i//4
        V_STREAM = v          # 40
        a_done = [i + 1 for i in range(NT)]
        A_STREAM = NT         # 32
        # final DVE ops: sum32 add, RB block reduces, 1 mul
        V_FINAL = V_STREAM + 1 + RB + 1
        A_FINAL = A_STREAM + 2

        @block.sync
        def _(sync):
            for i in range(NT):
                b, c = divmod(i, NCH)
                buf = i % NBUF
                if i >= NBUF:
                    sync.wait_ge(vsem, v_done[i - NBUF])
                    sync.wait_ge(asem, a_done[i - NBUF])
                sync.dma_start(
                    data[buf][:, :],
                    x_ext[b * P : (b + 1) * P, c * COLCH : (c + 1) * COLCH],
                ).then_inc(dmaL[buf], 16)

        @block.gpsimd
        def _(gpsimd):
            gpsimd.dma_start(
                yfb[:, :], yf_ext[0:1, :].to_broadcast([P, N])
            ).then_inc(dmaP, 16)
            gpsimd.dma_start(ylf_sb[:, :], ylf_ext[:, :]).then_inc(dmaP, 16)
            gpsimd.dma_start(off_sb[:, :], off_ext[:, :]).then_inc(dmaP, 16)
            gpsimd.wait_ge(dmaP, 48)
            x_flat = x_ext[:, :].rearrange("a b -> (a b)").unsqueeze(1)
            for b in range(RB):
                gpsimd.indirect_dma_start(
                    out=tv[:, b : b + 1],
                    out_offset=None,
                    in_=x_flat,
                    in_offset=bass.IndirectOffsetOnAxis(
                        ap=off_sb[:, b : b + 1], axis=0
                    ),
                ).then_inc(dmaG, 16)
            gpsimd.wait_ge(asem, A_FINAL)
            gpsimd.dma_start(out_ext[:, :], acc[:, :]).then_inc(dmaG, 16)

        @block.vector
        def _(vector):
            first_count = True
            for i in range(NT):
                buf = i % NBUF
                vector.wait_ge(dmaL[buf], 16 * (i // NBUF + 1))
                nc.vector.reduce_sum(
                    rs_part[:, i : i + 1],
                    data[buf][:, 0:D_DVE],
                    axis=mybir.AxisListType.X,
                ).then_inc(vsem, 1)
                if i % 4 == 3:
                    b = i // 4
                    if first_count:
                        vector.wait_ge(dmaP, 48)
                        first_count = False
                    nc.vector.tensor_scalar(
                        out=eqscr[:, :],
                        in0=yfb[:, :],
                        scalar1=ylf_sb[:, b : b + 1],
                        scalar2=None,
                        op0=mybir.AluOpType.is_equal,
                        op1=mybir.AluOpType.add,
                        accum_out=w_sb[:, b : b + 1],
                    ).then_inc(vsem, 1)
            # epilogue
            vector.wait_ge(asem, A_STREAM)
            nc.vector.tensor_tensor(
                out=sum32[:, :],
                in0=rs_part[:, :],
                in1=act_part[:, :],
                op=mybir.AluOpType.add,
            ).then_inc(vsem, 1)
            for b in range(RB):
                nc.vector.reduce_sum(
                    rs[:, b : b + 1],
                    sum32[:, b * NCH : (b + 1) * NCH],
                    axis=mybir.AxisListType.X,
                ).then_inc(vsem, 1)
            vector.wait_ge(dmaG, 16 * RB)
            nc.vector.tensor_tensor(
                out=tprod[:, :], in0=tv[:, :], in1=w_sb[:, :],
                op=mybir.AluOpType.mult,
            ).then_inc(vsem, 1)

        @block.scalar
        def _(scalar):
            for i in range(NT):
                buf = i % NBUF
                scalar.wait_ge(dmaL[buf], 16 * (i // NBUF + 1))
                nc.scalar.activation(
                    out=act_scr[:, :],
                    in_=data[buf][:, D_DVE:COLCH],
                    func=mybir.ActivationFunctionType.Copy,
                    accum_out=act_part[:, i : i + 1],
                ).then_inc(asem, 1)
            scalar.wait_ge(vsem, V_FINAL)
            nc.scalar.activation(
                out=logt[:, :],
                in_=tprod[:, :],
                func=mybir.ActivationFunctionType.Ln,
                accum_out=acc[:, 0:1],
            ).then_inc(asem, 1)
            nc.scalar.activation(
                out=logt[:, :],
                in_=rs[:, :],
                func=mybir.ActivationFunctionType.Ln,
                accum_out=acc[:, 1:2],
            ).then_inc(asem, 1)

    return nc


def _get_nc():
    if "nc" not in _CACHE:
        _CACHE["nc"] = _build_nc()
    return _CACHE["nc"]


def kernel(output, y):
    from concourse.bass_utils import run_bass_kernel_spmd

    output = np.asarray(output)
    y = np.asarray(y)
    assert output.shape == (N, C) and y.shape == (N,)
    out_f32 = np.ascontiguousarray(output, dtype=np.float32)
    y64 = y.astype(np.int64)

    yf = y64.astype(np.float32).reshape(1, N)
    in_maps = []
    for k in range(NCORES):
        rows = slice(k * RPC, (k + 1) * RPC)
        y_loc = y64[rows]
        # (p, b) layout: element (p, b) corresponds to local row b*128 + p
        ylf = np.ascontiguousarray(
            y_loc.astype(np.float32).reshape(RB, P).T
        )
        off = np.ascontiguousarray(
            (np.arange(RPC, dtype=np.int64) * C + y_loc)
            .astype(np.int32)
            .reshape(RB, P)
            .T
        )
        in_maps.append(
            {"x": out_f32[rows], "yf": yf, "ylf": ylf, "off": off}
        )

    res = run_bass_kernel_spmd(
        _get_nc(), in_maps, core_ids=list(range(NCORES))
    )
    total = 0.0
    for k in range(NCORES):
        o = res.results[k]["out"]
        total += float(o[:, 0].sum(dtype=np.float64)) - float(
            o[:, 1].sum(dtype=np.float64)
        )
    loss = math.log(N) - total / N
    return np.float32(loss)
